# revision 56
# baseline (speedup 1.0000x reference)
"""Segment-sum (scatter-add) kernel for Trainium2, SPMD over 8 NeuronCores.

Problem: out[n, :] = sum over edges e with X_node[e] == n of H[e, :]
  H [E=800000, 64] f32, X_node [E] int64, node_num N=50000 -> out [N, 64] f32.

Strategy (v2)
-------------
Host-side sharding: edges are bucketed by destination node (each core owns a
contiguous node range chosen so per-core edge counts are ~equal).  Within a
core, nodes are greedily packed into "windows" of <= WN=16 consecutive nodes
whose edges fit in B blocks of 128 edges; every window is padded to exactly
B*128 edge slots so all 8 cores run one identical SPMD program.

Per 128-edge block the host ships 144 B/partition: the edge's H row as bf16
(128 B) plus a one-hot fp8 mask [128 edges x WN nodes] (16 B).  The kernel is
HBM-bandwidth bound, so bytes are king: 144 B/edge total vs 224 for the old
3-way fp8 cascade.  bf16 quantization gives rel-err ~2e-3 (measured on the
actual data), well inside the 2e-2 gate.

Device kernel per core (one matmul per block, no fold pipeline):
  PE:  psum[WN, 64] (+)= mask.T @ H_bf16 -- mixed fp8 lhsT x bf16 rhs matmul,
       B blocks accumulate per window; G_PS=8 windows share one PSUM bank,
       pool of 4 banks ping-pongs.
  ACT/DVE: alternate groups copy psum -> sbuf f32 (no arithmetic needed).
  DMA: sync ring streams packed chunks (ramped sizes so the first matmul
       starts after ~0.2 MB), gpsimd SWDGE ring does stores so they never
       queue ahead of loads.
Host gathers window rows into out[n0:n1, :] (pure layout, no arithmetic).
"""

import os

import numpy as np
import ml_dtypes

BF16 = np.dtype(ml_dtypes.bfloat16)
FP8 = np.dtype(ml_dtypes.float8_e4m3)

N_CORES = 8
P = 128
D = 64
WN = 32    # nodes per window (mask width)
G_PS = 8   # windows per PSUM bank
CH = 32    # steady-state blocks per DMA chunk


def _chunk_plan(T):
    """Chunk sizes (in blocks) ramp 4,4,8,16,32 then CH, ramping back down
    at the end: the first matmul only waits for a small chunk, and the
    drain/store tail after the last byte lands stays short."""
    head = []
    t = 0
    for s in (4, 4, 8, 16, 32):
        if t + s > T:
            break
        head.append(s)
        t += s
    tail = []
    for s in (8, 16):
        if t + s > T:
            break
        tail.append(s)
        t += s
    sizes = []
    while t < T:
        s = min(CH, T - t)
        sizes.append(s)
        t += s
    return head + sizes + tail[::-1]


def _schedule(W, B):
    """Stream order of (window, block) pairs.  Consecutive matmuls cycle
    through the 4 PSUM column-tile positions so each LDWEIGHTS targets a
    different 32-col PE subarray than the in-flight matmul - they overlap
    instead of serializing (~2x PE throughput)."""
    WT = 4 * G_PS
    NT = -(-W // WT)
    sched = []
    for k in range(NT):
        for g in range(G_PS):
            for b in range(B):
                for q in range(4):
                    w = k * WT + q * G_PS + g
                    if w < W:
                        sched.append((w, b))
    return sched


# ----------------------------------------------------------------- planning
def _pack_windows(counts, n0, n1, B):
    """Greedily pack nodes [n0, n1) into windows of <=WN nodes whose total
    edge count fits in B*128 slots.  Returns list of (node_start, n_nodes)."""
    cap = B * P
    wins = []
    ws = n0
    acc = 0
    nn = 0
    for n in range(n0, n1):
        c = int(counts[n])
        if nn == WN or (acc + c > cap and nn > 0):
            wins.append((ws, nn))
            ws, acc, nn = n, 0, 0
        if c > cap:
            return None  # single node exceeds capacity; need bigger B
        acc += c
        nn += 1
    if nn > 0:
        wins.append((ws, nn))
    return wins


def _plan(X, N):
    """Choose core node ranges, B (blocks/window) and W (windows/core)."""
    E = X.shape[0]
    order = np.argsort(X, kind="stable")
    Xs = X[order]
    counts = np.bincount(X, minlength=N)
    cum = np.zeros(N + 1, dtype=np.int64)
    np.cumsum(counts, out=cum[1:])

    nb = [0]
    for c in range(1, N_CORES):
        nb.append(int(np.searchsorted(cum, round(E * c / N_CORES), side="left")))
    nb.append(N)

    b_lo = max(1, -(-int(counts.max()) // P))
    best = None
    for B in range(b_lo, b_lo + 12):
        wins_all = []
        ok = True
        for c in range(N_CORES):
            wins = _pack_windows(counts, nb[c], nb[c + 1], B)
            if wins is None:
                ok = False
                break
            wins_all.append(wins)
        if not ok:
            continue
        W = max(len(w) for w in wins_all)
        cost = W * B  # proportional to padded edges (dominant DMA)
        if best is None or cost < best[0]:
            best = (cost, B, W, wins_all)
    assert best is not None, "window packing failed"
    _, B, W, wins_all = best
    return order, Xs, cum, nb, B, W, wins_all


def _build_core_inputs(H32, order, Xs, cum, wins, B, W):
    """Build the padded, reordered device input bytes for one core."""
    T = W * B
    # per-window slots first (window-major), then permute into stream order
    widx = np.full((W, B * P), -1, dtype=np.int64)
    woff = np.full((W, B * P), 255, dtype=np.int64)  # >= WN: all-zero mask
    for w, (ns, nn) in enumerate(wins):
        e0 = int(cum[ns])
        e1 = int(cum[ns + nn])
        ec = e1 - e0
        widx[w, :ec] = order[e0:e1]
        woff[w, :ec] = Xs[e0:e1] - ns

    sched = _schedule(W, B)
    assert len(sched) == T
    idx = np.empty((T, P), dtype=np.int64)
    off = np.empty((T, P), dtype=np.int64)
    for t, (w, b) in enumerate(sched):
        idx[t] = widx[w, b * P : (b + 1) * P]
        off[t] = woff[w, b * P : (b + 1) * P]

    Hg = H32[np.maximum(idx, 0)]
    Hg[idx < 0] = 0.0
    hb = Hg.astype(BF16).view(np.uint8).reshape(T, P, 2 * D)
    pkt = np.ascontiguousarray(hb.transpose(1, 0, 2).reshape(P, T * 2 * D))
    offt = np.ascontiguousarray(off.T.astype(BF16))  # [P, T]
    return pkt.view(FP8), offt


# ------------------------------------------------------------- device kernel
def _build_program(T, W, B):
    import concourse.bacc as bacc
    import concourse.tile as tile
    import concourse.mybir as mybir

    nc = bacc.Bacc("TRN2", target_bir_lowering=False, debug=False)
    fp8 = mybir.dt.float8e4
    bf16 = mybir.dt.bfloat16
    f32 = mybir.dt.float32

    PKW = 2 * D  # packed bytes per block per partition: H bf16
    with tile.TileContext(nc) as tc:
        with tc.tile_pool(name="dram", bufs=1, space="DRAM") as dram:
            # windows pack 4-deep along PSUM partitions (col tiling at
            # offsets 0/32/64/96) and G_PS-deep along the free dim: one
            # PSUM bank holds WT = 4*G_PS windows, so drains and stores
            # happen once per WT windows instead of once per G_PS.
            WT = 4 * G_PS
            NT = -(-W // WT)  # psum tiles (= drains = stores)
            pkt = dram.tile([P, T * PKW], fp8, kind="ExternalInput")
            # per-edge window offsets, one bf16 per (partition, block):
            # loaded once at start so mask-gen never waits on the H stream
            offt = dram.tile([P, T], bf16, kind="ExternalInput")
            # replicated iota constant for device-side mask generation:
            # iota_d[p, c, j] = j
            iota_d = dram.tile([P, CH * WN], bf16, kind="ExternalInput")
            # tile-major [NT*128, G_PS*D] bf16 so each psum-tile store is
            # one contiguous 128 KiB DRAM block; window w -> row
            # (w//WT)*128 + 32*((w%WT)//G_PS) + node_off, col (w%G_PS)*D
            odev = dram.tile([NT * 128, G_PS * D], bf16, kind="ExternalOutput")

            with tc.tile_pool(name="hbuf", bufs=16) as hpool, \
                 tc.tile_pool(name="mbuf", bufs=16) as mpool, \
                 tc.tile_pool(name="cbuf", bufs=1) as cpool, \
                 tc.tile_pool(name="psum", bufs=6, space="PSUM") as pspool, \
                 tc.tile_pool(name="wps", bufs=1, space="PSUM") as wpool, \
                 tc.tile_pool(name="outb", bufs=1) as opool:

                # tiny startup constants (loaded right after the first H
                # chunk is issued): iota + the first chunks' offs on the
                # sync ring, the bulk of the offs on the gpsimd ring
                OFF1 = min(4 * CH, T)
                iota = cpool.tile([P, CH, WN], bf16)
                offsb = cpool.tile([P, T], bf16)

                def _load_consts():
                    # gpsimd ring: empty at start (stores come much later),
                    # so const receipts never serialize behind H chunks
                    nc.gpsimd.dma_start(
                        out=iota[:],
                        in_=iota_d[:].rearrange("p (c j) -> p c j", c=CH),
                    )
                    nc.gpsimd.dma_start(out=offsb[:, :OFF1], in_=offt[:, :OFF1])
                    if OFF1 < T:
                        nc.gpsimd.dma_start(
                            out=offsb[:, OFF1:], in_=offt[:, OFF1:]
                        )

                # one static output staging tile; drains write disjoint
                # slices (subtile deps), stores read them - no WAR ring
                obig = opool.tile([128, NT, G_PS * D], bf16)

                # ~4us of serial dummy matmuls on a memset scratch: PE
                # warms its HAM clock gate (1.2 -> 2.4 GHz) from ~program
                # start, no DMA dependency; mid-stream idle stints are too
                # short to re-throttle it
                wsrc = cpool.tile([P, 512], bf16)
                nc.vector.memset(wsrc[:], 1.0)
                wps = wpool.tile([WN, 512], f32)
                for _ in range(10):
                    nc.tensor.matmul(
                        out=wps[:],
                        lhsT=wsrc[:, :WN],
                        rhs=wsrc[:],
                        start=True,
                        stop=True,
                    )

                chunk_starts = {}
                t_acc = 0
                for s in _chunk_plan(T):
                    chunk_starts[t_acc] = s
                    t_acc += s

                sched = _schedule(W, B)
                pk = None
                msk = None
                t0 = 0
                ps = None
                cur_k = -1
                n_chunk = 0
                for t, (w, b) in enumerate(sched):
                    g = w % G_PS
                    q = (w % WT) // G_PS  # psum column-tile position
                    k = w // WT
                    if k != cur_k:
                        ps = pspool.tile([128, G_PS, D], f32)
                        cur_k = k
                    if t in chunk_starts:
                        ch = chunk_starts[t]
                        t0 = t
                        pk = hpool.tile([P, CH, PKW], fp8, tag="h")
                        # alternate between the two HWDGE rings (SP/ACT)
                        # so one ring's per-DMA handshake hides behind
                        # the other's transfer
                        ring = nc.sync if n_chunk % 2 == 0 else nc.scalar
                        n_chunk += 1
                        ring.dma_start(
                            out=pk[:, :ch, :],
                            in_=pkt[:, t * PKW : (t + ch) * PKW].rearrange(
                                "p (c d) -> p c d", c=ch
                            ),
                        )
                        if t == 0:
                            _load_consts()
                        # device-side one-hot masks from the resident
                        # off tile: decoupled from the H stream, DVE
                        # can run many chunks ahead
                        msk = mpool.tile([P, CH, WN], bf16, tag="m")
                        s0 = 0
                        while s0 < ch:
                            s1 = min(s0 + 32, ch)
                            nc.vector.tensor_tensor(
                                out=msk[:, s0:s1, :],
                                in0=iota[:, : s1 - s0, :],
                                in1=offsb[:, t + s0 : t + s1, None].to_broadcast(
                                    [P, s1 - s0, WN]
                                ),
                                op=mybir.AluOpType.is_equal,
                            )
                            s0 = s1
                    rel = t - t0
                    nc.tensor.matmul(
                        out=ps[32 * q : 32 * q + WN, g, :],
                        lhsT=msk[:, rel, :],
                        rhs=pk[:, rel, :].bitcast(bf16),
                        start=(b == 0),
                        stop=(b == B - 1),
                        tile_position=(0, 32 * q),
                    )
                    if t + 1 == len(sched) or sched[t + 1][0] // WT != k:
                        # drains all on ACT: DVE is busy with mask-gen
                        nc.scalar.mul(
                            out=obig[:, k, :], in_=ps[:, :, :], mul=1.0
                        )
                        # HWDGE stores spread across all 16 engines
                        # (SWDGE concentrated them on engine 0)
                        sring = nc.sync if k % 2 == 0 else nc.scalar
                        sring.dma_start(
                            out=odev[k * 128 : (k + 1) * 128, :],
                            in_=obig[:, k, :],
                        )
    nc.compile()
    return nc, pkt, offt, iota_d, odev


# --------------------------------------------------------------------- main
def kernel(H, X_node, node_num):
    from concourse import bass_utils

    H32 = np.asarray(H, dtype=np.float32)
    X = np.asarray(X_node).astype(np.int64)
    N = int(node_num)
    E = X.shape[0]
    assert H32.shape == (E, D)

    order, Xs, cum, nb, B, W, wins_all = _plan(X, N)
    T = W * B

    nc, pkt, offt, iota_d, odev = _build_program(T, W, B)
    iota_np = np.ascontiguousarray(
        np.broadcast_to(np.arange(WN, dtype=BF16), (P, CH, WN)).reshape(P, CH * WN)
    )
    in_maps = []
    for c in range(N_CORES):
        pkt_np, offt_np = _build_core_inputs(H32, order, Xs, cum, wins_all[c], B, W)
        in_maps.append(
            {pkt.name: pkt_np, offt.name: offt_np, iota_d.name: iota_np}
        )

    trace = bool(int(os.environ.get("SEGSUM_TRACE", "0")))
    res = bass_utils.run_bass_kernel_spmd(
        nc, in_maps, core_ids=list(range(N_CORES)), trace=trace
    )
    if trace:
        kernel.last_exec_time_ns = res.exec_time_ns
        kernel.last_mean_exec_time_ns = res.mean_exec_time_ns
        kernel.last_trace = (
            res.instructions_and_trace[1] if res.instructions_and_trace else None
        )

    WT = 4 * G_PS
    NT = -(-W // WT)
    out = np.zeros((N, D), dtype=np.float32)
    for c in range(N_CORES):
        ot = np.asarray(res.results[c][odev.name])
        if ot.dtype != np.float32:
            ot = ot.view(BF16) if ot.dtype.itemsize == 2 else ot
        # [tile, partition, group, D]: window w lives at tile w//WT,
        # partition 32*((w%WT)//G_PS)+node_off, group w%G_PS
        ot = ot.astype(np.float32).reshape(NT, 128, G_PS, D)
        for w, (ns, nn) in enumerate(wins_all[c]):
            q = (w % WT) // G_PS
            out[ns : ns + nn, :] = ot[w // WT, 32 * q : 32 * q + nn, w % G_PS, :]
    return out
